# revision 2
# baseline (speedup 1.0000x reference)
"""Cross-attention kernel for Trainium2, 8 NeuronCores.

Sharding: batch (4) x head-group (2 groups of 8 heads) = 8 cores.
Per core: q/k/v projections for its 8 heads, attention, PV, and a partial
output projection (its 512 context channels); host sums the two per-batch
partials and adds the bias.

Layout strategy (everything contracts on the partition dim):
  xT/yT [C,N] host-pretransposed -> qT/kT computed as [dq, Nx]/[dk, Ny]
  (head-dim on partitions), v natural [Ny, dv].
  Natural side S=[Nx,Ny]: softmax along free dim, exp fused with rowsum on
  ACT, attn=(E*recip)*W in one DVE op -> DMA out.
  Transposed side ST=[Ny,Nx]: exp -> *W.T -> PV with v-as-stationary gives
  ctxT [dv, Nx]; normalization folded after PV via recip rows broadcast
  from a DRAM bounce (per-head PE transpose of the recip columns).
"""
import numpy as np
from contextlib import ExitStack

import concourse.bass as bass
import concourse.bacc as bacc
import concourse.mybir as mybir
import concourse.tile as tile
from concourse.bass_utils import run_bass_kernel_spmd
from concourse.masks import make_identity
import ml_dtypes

F32 = mybir.dt.float32
BF16 = mybir.dt.bfloat16
AF = mybir.ActivationFunctionType
OP = mybir.AluOpType

B, NX, NY, DIM = 4, 1024, 1024, 1024
HEADS, HD = 16, 64
HG = 8                      # heads per core
DQ = HG * HD                # 512 local q/k/v channels
SCALE = HD ** -0.5
NCORES = 8

_CACHE = {}


def _build():
    nc = bacc.Bacc("TRN2", debug=False)

    xT = nc.declare_dram_parameter("xT", [DIM, NX], BF16, isOutput=False)
    yT = nc.declare_dram_parameter("yT", [DIM, NY], BF16, isOutput=False)
    wq = nc.declare_dram_parameter("wq", [DIM, DQ], BF16, isOutput=False)
    wk = nc.declare_dram_parameter("wk", [DIM, DQ], BF16, isOutput=False)
    wv = nc.declare_dram_parameter("wv", [DIM, DQ], BF16, isOutput=False)
    wp = nc.declare_dram_parameter("wp", [DQ, DIM], BF16, isOutput=False)
    wf = nc.declare_dram_parameter("wf", [NX, NY], F32, isOutput=False)
    wtb = nc.declare_dram_parameter("wtb", [NY, NX], BF16, isOutput=False)
    attn_o = nc.declare_dram_parameter("attn_o", [HG, NX, NY], F32, isOutput=True)
    p_o = nc.declare_dram_parameter("p_o", [NX, DIM], F32, isOutput=True)

    with tile.TileContext(nc) as tc, ExitStack() as ctx:
        wpool = ctx.enter_context(tc.tile_pool(name="wpool", bufs=1))
        qkv = ctx.enter_context(tc.tile_pool(name="qkv", bufs=1))
        dram = ctx.enter_context(tc.tile_pool(name="dram", bufs=2, space="DRAM"))

        # ---- load weights ----
        wq_t = [wpool.tile([128, DQ], BF16, tag=f"wq{c}", name=f"wq{c}") for c in range(8)]
        wk_t = [wpool.tile([128, DQ], BF16, tag=f"wk{c}", name=f"wk{c}") for c in range(8)]
        wv_t = [wpool.tile([128, DQ], BF16, tag=f"wv{c}", name=f"wv{c}") for c in range(8)]
        wp_t = [wpool.tile([128, DIM], BF16, tag=f"wp{c}", name=f"wp{c}") for c in range(4)]
        for c in range(8):
            nc.sync.dma_start(out=wq_t[c], in_=wq[c * 128:(c + 1) * 128, :])
            nc.sync.dma_start(out=wk_t[c], in_=wk[c * 128:(c + 1) * 128, :])
            nc.sync.dma_start(out=wv_t[c], in_=wv[c * 128:(c + 1) * 128, :])
        for c in range(4):
            nc.sync.dma_start(out=wp_t[c], in_=wp[c * 128:(c + 1) * 128, :])
        ident = wpool.tile([128, 128], F32, tag="ident")
        make_identity(nc, ident)

        # persistent q/k/v/ctx
        qT = [qkv.tile([128, NX], BF16, tag=f"qT{m}", name=f"qT{m}") for m in range(4)]
        kT = [qkv.tile([128, NY], BF16, tag=f"kT{m}", name=f"kT{m}") for m in range(4)]
        v_t = [qkv.tile([128, DQ], BF16, tag=f"v{j}", name=f"v{j}") for j in range(8)]
        ctxT = [qkv.tile([128, NX], BF16, tag=f"ctx{m}", name=f"ctx{m}") for m in range(4)]

        # ---- phase A: projections ----
        with ExitStack() as actx:
            xpool = actx.enter_context(tc.tile_pool(name="xpool", bufs=1))
            psq = actx.enter_context(tc.tile_pool(name="psq", bufs=4, space="PSUM"))
            xT_t = [xpool.tile([128, NX], BF16, tag=f"xT{c}", name=f"xT{c}") for c in range(8)]
            yT_t = [xpool.tile([128, NY], BF16, tag=f"yT{c}", name=f"yT{c}") for c in range(8)]
            for c in range(8):
                nc.sync.dma_start(out=xT_t[c], in_=xT[c * 128:(c + 1) * 128, :])
                nc.sync.dma_start(out=yT_t[c], in_=yT[c * 128:(c + 1) * 128, :])

            for m in range(4):          # qT[m] = (Wq x)[m-block]  [128dq, NX]
                for n in range(2):
                    ps = psq.tile([128, 512], F32, tag="ps")
                    for c in range(8):
                        nc.tensor.matmul(
                            ps, wq_t[c][:, m * 128:(m + 1) * 128],
                            xT_t[c][:, n * 512:(n + 1) * 512],
                            start=(c == 0), stop=(c == 7))
                    nc.vector.tensor_copy(qT[m][:, n * 512:(n + 1) * 512], ps)
            for m in range(4):
                for n in range(2):
                    ps = psq.tile([128, 512], F32, tag="ps")
                    for c in range(8):
                        nc.tensor.matmul(
                            ps, wk_t[c][:, m * 128:(m + 1) * 128],
                            yT_t[c][:, n * 512:(n + 1) * 512],
                            start=(c == 0), stop=(c == 7))
                    nc.vector.tensor_copy(kT[m][:, n * 512:(n + 1) * 512], ps)
            for j in range(8):          # v[j] = y[j-block] @ Wv.T  [128ny, 512]
                ps = psq.tile([128, 512], F32, tag="ps")
                for c in range(8):
                    nc.tensor.matmul(
                        ps, yT_t[c][:, j * 128:(j + 1) * 128], wv_t[c],
                        start=(c == 0), stop=(c == 7))
                nc.vector.tensor_copy(v_t[j], ps)

        # ---- phase B: attention, weights resident ----
        with ExitStack() as bctx:
            wfp = bctx.enter_context(tc.tile_pool(name="wfp", bufs=1))
            work = bctx.enter_context(tc.tile_pool(name="work", bufs=3))
            ewp = bctx.enter_context(tc.tile_pool(name="ewp", bufs=2))
            psS = bctx.enter_context(tc.tile_pool(name="psS", bufs=2, space="PSUM"))
            psO = bctx.enter_context(tc.tile_pool(name="psO", bufs=2, space="PSUM"))
            psT = bctx.enter_context(tc.tile_pool(name="psT", bufs=2, space="PSUM"))

            wf_t = [wfp.tile([128, NY], F32, tag=f"wf{i}", name=f"wf{i}") for i in range(8)]
            wtb_t = [wfp.tile([128, NX], BF16, tag=f"wtb{j}", name=f"wtb{j}") for j in range(8)]
            for i in range(8):
                nc.sync.dma_start(out=wf_t[i], in_=wf[i * 128:(i + 1) * 128, :])
                nc.sync.dma_start(out=wtb_t[i], in_=wtb[i * 128:(i + 1) * 128, :])

            for h in range(HG):
                qh = qT[h // 2][(h % 2) * 64:(h % 2) * 64 + 64, :]
                kh = kT[h // 2][(h % 2) * 64:(h % 2) * 64 + 64, :]

                # natural side: S=[Nx,Ny], softmax over free dim, attn out
                rcol = work.tile([128, 8], F32, tag="rcol")
                for i in range(8):
                    pss = psS.tile([128, 1024], F32, tag="pss")
                    for n in range(2):
                        nc.tensor.matmul(
                            pss[:, n * 512:(n + 1) * 512],
                            qh[:, i * 128:(i + 1) * 128],
                            kh[:, n * 512:(n + 1) * 512],
                            start=True, stop=True)
                    e_t = work.tile([128, 1024], F32, tag="e")
                    rs = work.tile([128, 1], F32, tag="rs")
                    nc.scalar.activation(e_t, pss, AF.Exp, scale=SCALE,
                                         accum_out=rs)
                    nc.vector.reciprocal(rcol[:, i:i + 1], rs)
                    at = work.tile([128, 1024], F32, tag="at")
                    nc.vector.scalar_tensor_tensor(
                        out=at, in0=e_t, scalar=rcol[:, i:i + 1], in1=wf_t[i],
                        op0=OP.mult, op1=OP.mult)
                    nc.sync.dma_start(
                        out=attn_o[h, i * 128:(i + 1) * 128, :], in_=at)

                # recip rows: [128,8] -> PE transpose -> DRAM -> bcast [64,1024]
                pst = psT.tile([8, 128], F32, tag="pst")
                nc.tensor.transpose(pst, rcol, ident[:, :])
                trs = work.tile([8, 128], F32, tag="trs")
                nc.vector.tensor_copy(trs, pst)
                rrow_d = dram.tile([1, 1024], F32, tag="rrow")
                nc.sync.dma_start(
                    out=rrow_d.rearrange("o (a b) -> (o a) b", a=8), in_=trs)
                bct = work.tile([64, 1024], F32, tag="bct")
                src = bass.AP(tensor=rrow_d.tensor, offset=rrow_d.offset,
                              ap=[[0, 64], [1, 1024]])
                nc.sync.dma_start(out=bct, in_=src)

                # transposed side: ST=[Ny,Nx] -> exp -> *W.T (bf16)
                ew = [ewp.tile([128, NX], BF16, tag=f"ew{j}", name=f"ew{j}") for j in range(8)]
                for j in range(8):
                    psst = psS.tile([128, 1024], F32, tag="pss")
                    for n in range(2):
                        nc.tensor.matmul(
                            psst[:, n * 512:(n + 1) * 512],
                            kh[:, j * 128:(j + 1) * 128],
                            qh[:, n * 512:(n + 1) * 512],
                            start=True, stop=True)
                    se = work.tile([128, 1024], BF16, tag="se")
                    nc.scalar.activation(se, psst, AF.Exp, scale=SCALE)
                    nc.vector.tensor_mul(ew[j], se, wtb_t[j])

                # PV: ctxT[h] = sum_j v[j][:,h].T @ EW[j], then *recip-bcast
                for n in range(2):
                    pso = psO.tile([64, 512], F32, tag="pso")
                    for j in range(8):
                        nc.tensor.matmul(
                            pso, v_t[j][:, h * 64:(h + 1) * 64],
                            ew[j][:, n * 512:(n + 1) * 512],
                            start=(j == 0), stop=(j == 7))
                    nc.vector.tensor_mul(
                        ctxT[h // 2][(h % 2) * 64:(h % 2) * 64 + 64,
                                     n * 512:(n + 1) * 512],
                        pso, bct[:, n * 512:(n + 1) * 512])

        # ---- phase C: partial output projection ----
        with ExitStack() as cctx:
            psP = cctx.enter_context(tc.tile_pool(name="psP", bufs=4, space="PSUM"))
            pop = cctx.enter_context(tc.tile_pool(name="pop", bufs=3))
            for i in range(8):
                for n in range(2):
                    ps = psP.tile([128, 512], F32, tag="psp")
                    for c4 in range(4):
                        nc.tensor.matmul(
                            ps, ctxT[c4][:, i * 128:(i + 1) * 128],
                            wp_t[c4][:, n * 512:(n + 1) * 512],
                            start=(c4 == 0), stop=(c4 == 3))
                    po = pop.tile([128, 512], F32, tag="po")
                    nc.vector.tensor_copy(po, ps)
                    nc.sync.dma_start(
                        out=p_o[i * 128:(i + 1) * 128, n * 512:(n + 1) * 512],
                        in_=po)

    nc.finalize()
    return nc


def _prep_inputs(x, y, weights, Wq, Wkv, Wp):
    bf = ml_dtypes.bfloat16
    in_maps = []
    for b in range(B):
        xTb = np.ascontiguousarray(x[b].T).astype(bf)
        yTb = np.ascontiguousarray(y[b].T).astype(bf)
        wfb = np.ascontiguousarray(weights[b, 0])
        wtbb = np.ascontiguousarray(weights[b, 0].T).astype(bf)
        for g in range(2):
            h0 = g * HG * HD
            in_maps.append({
                "xT": xTb,
                "yT": yTb,
                "wq": np.ascontiguousarray(Wq[h0:h0 + DQ, :].T).astype(bf),
                "wk": np.ascontiguousarray(Wkv[h0:h0 + DQ, :].T).astype(bf),
                "wv": np.ascontiguousarray(
                    Wkv[DIM + h0:DIM + h0 + DQ, :].T).astype(bf),
                "wp": np.ascontiguousarray(
                    Wp[:, h0:h0 + DQ].T).astype(bf),
                "wf": wfb,
                "wtb": wtbb,
            })
    return in_maps


def _gather(results, bp):
    out = np.empty((B, NX, DIM), np.float32)
    attn = np.empty((B, HEADS, NX, NY), np.float32)
    for b in range(B):
        out[b] = results[2 * b]["p_o"] + results[2 * b + 1]["p_o"] + bp
        attn[b, :HG] = results[2 * b]["attn_o"]
        attn[b, HG:] = results[2 * b + 1]["attn_o"]
    return out, attn


def _run(x, y, weights, Wq, Wkv, Wp, bp, trace=False, tmpdir=None):
    if "nc" not in _CACHE:
        _CACHE["nc"] = _build()
    nc = _CACHE["nc"]
    in_maps = _prep_inputs(x, y, weights, Wq, Wkv, Wp)
    res = run_bass_kernel_spmd(nc, in_maps, list(range(NCORES)),
                               trace=trace, tmpdir=tmpdir)
    out, attn = _gather(res.results, np.asarray(bp, np.float32))
    return (out, attn), res


def kernel(x, y, weights, Wq, Wkv, Wp, bp):
    x = np.asarray(x, np.float32)
    y = np.asarray(y, np.float32)
    weights = np.asarray(weights, np.float32)
    Wq = np.asarray(Wq, np.float32)
    Wkv = np.asarray(Wkv, np.float32)
    Wp = np.asarray(Wp, np.float32)
    bp = np.asarray(bp, np.float32)
    (out, attn), _ = _run(x, y, weights, Wq, Wkv, Wp, bp, trace=False)
    return out, attn
